# revision 13
# baseline (speedup 1.0000x reference)
"""Trainium2 Bass kernel for MultiLatentAttention (MLA).

Sharding: 8 cores = 2 (batch) x 4 (head-groups of 4 heads).
Within each batch group of 4 cores, the down-projections are sharded by
output rows and AllGathered (per S-panel), the shared k_rope head is
sharded by S-panel and gathered once early.  Each core then runs its 4
heads' up-projections + SDPA and a partial output projection
y_part = attn_out @ Wo[:, heads].T.  Host sums the 4 partials per batch.

On-device layout is feature-major ("transposed"): activations are [feat, S]
so every matmul contracts along the partition dim with zero transposes.
Scores are computed transposed [k, q].

Phase C (SDPA) is software-pipelined one block deep: the score matmuls of
block i+1 are emitted between the scores and the attn@v matmul of block i,
hiding the exp (Act engine) latency.  The causal mask is applied as a PE
bias-matmul (identity x (-30000 * triangle)) accumulated into the score
psum, diagonal blocks only compute their unmasked column range, and the
softmax denominator comes from a bf16 running sum of exp tiles on DVE
followed by a single ones-vector matmul per head.  Attention output stays
in SBUF (no DRAM roundtrip) and feeds the output projection, whose chunks
are interleaved into the next panel's block stream as PE fillers.
All matmul operands are bf16 (f32 PSUM accumulation).
"""

import sys

if "/opt/trn_rl_repo" not in sys.path:
    sys.path.insert(0, "/opt/trn_rl_repo")

import numpy as np
import ml_dtypes

BF16 = ml_dtypes.bfloat16

B, S, D, H = 2, 2048, 2048, 16
QR, KVR = 1536, 512
NOPE, RD, VD = 128, 64, 128
QK_D = NOPE + RD
HL = 4          # heads per core
G = 4           # head groups (= cores per batch group)
QSH = QR // G   # 384 c_q rows per core
KSH = KVR // G  # 128 c_kv rows per core
PAN = 512       # panel width
P = 128

_cache = {}
_marks = []


def _mark(nc, label):
    _marks.append((label, int(nc.get_next_instruction_name().split("-")[1])))


def _build_module(reps=1, phases="ABCD"):
    import concourse.bacc as bacc
    import concourse.mybir as mybir
    import concourse.tile as tile

    dt = mybir.dt
    f32, bf16 = dt.float32, dt.bfloat16
    AF = mybir.ActivationFunctionType

    nc = bacc.Bacc("TRN2", target_bir_lowering=False, debug=False, num_devices=8)

    def inp(name, shape, dtype=bf16):
        return nc.dram_tensor(name, shape, dtype, kind="ExternalInput").ap()

    xT = inp("xT", [D, S])                  # x[b].T
    xkr = inp("xkr", [D, PAN])              # x[b].T[:, my panel]
    # weights are host-packed to the on-chip [128, kt*m] layout so every
    # DMA is fully contiguous (no <512B-element half-speed penalty)
    wqd = inp("wqd", [P, (D // P) * QSH])   # Wq_down.T column slice
    wkvd = inp("wkvd", [P, (D // P) * KSH]) # Wkv_down.T column slice
    wkr = inp("wkr", [P, (D // P) * RD])    # Wk_rope.T
    wqall = inp("wqall", [P, (QR // P) * 768])  # [Wq_up_g.T*s | Wq_rope_g.T*s]
    wku = inp("wku", [P, (KVR // P) * 512]) # Wk_up_g.T
    wvu = inp("wvu", [P, (KVR // P) * 512]) # Wv_up_g.T
    wo = inp("wo", [P, 4 * D])              # Wo[:, cols_g].T
    cosT = inp("cosT", [32, S], f32)
    sinT = inp("sinT", [32, S], f32)
    coskr = inp("coskr", [32, PAN], f32)    # cos/sin for my k_rope panel
    sinkr = inp("sinkr", [32, PAN], f32)
    ident = inp("ident", [P, P])            # identity (bf16)
    trib = inp("trib", [P, P])              # -30000 where p > q else 0 (bf16)
    onc = inp("onc", [P, 1])                # ones column
    y = nc.dram_tensor("y", [S, D], f32, kind="ExternalOutput").ap()

    KT_D = D // P      # 16 k-tiles over model dim
    KT_QR = QR // P    # 12
    KT_KV = KVR // P   # 4
    NP = S // PAN      # 4 panels
    GROUPS = [[0, 1, 2, 3], [4, 5, 6, 7]]

    with tile.TileContext(nc) as tc:
      for _rep in range(reps):
        with (
            tc.tile_pool(name="res", bufs=1) as res,
            tc.tile_pool(name="panels", bufs=8) as panels,
            tc.tile_pool(name="work", bufs=2) as work,
            tc.tile_pool(name="dram", bufs=1, space="DRAM") as dram,
        ):
            # ---- SBUF residents for SDPA --------------------------------
            qn_sb = res.tile([P, HL, S], bf16, tag="qn")
            qr_sb = res.tile([64, HL, S], bf16, tag="qr")
            k_c_sb = res.tile([P, HL, S], bf16, tag="k_c")
            v_sb = res.tile([P, S // P, 512], bf16, tag="v")
            k_r_sb = res.tile([64, NP, PAN], bf16, tag="k_r")
            ident_sb = res.tile([P, P], bf16, tag="ident")
            trib_sb = res.tile([P, P], bf16, tag="trib")
            onc_sb = res.tile([P, 1], bf16, tag="onc")

            # ---- DRAM staging -------------------------------------------
            ag_in = [dram.tile([QSH + KSH, PAN], bf16, tag=f"agi{n}", name=f"agi{n}")
                     for n in range(NP)]
            ag_out = [dram.tile([G * (QSH + KSH), PAN], bf16, tag=f"ago{n}",
                                name=f"ago{n}") for n in range(NP)]
            kr_in = dram.tile([64, PAN], bf16, tag="kri", name="kri")
            kr_out = dram.tile([G * 64, PAN], bf16, tag="kro", name="kro")

            def rope_block(dst64, src64, cs, sn):
                # dst/src are [64, PAN]; rows 0:32 = first half dims
                t1 = work.tile([32, PAN], f32, tag="rope_t1", bufs=1)
                t2 = work.tile([32, PAN], f32, tag="rope_t2", bufs=1)
                nc.vector.tensor_mul(t1, src64[0:32, :], cs)
                nc.vector.tensor_mul(t2, src64[32:64, :], sn)
                nc.vector.tensor_sub(dst64[0:32, :], t1, t2)
                t3 = work.tile([32, PAN], f32, tag="rope_t1", bufs=1)
                t4 = work.tile([32, PAN], f32, tag="rope_t2", bufs=1)
                nc.vector.tensor_mul(t3, src64[32:64, :], cs)
                nc.vector.tensor_mul(t4, src64[0:32, :], sn)
                nc.vector.tensor_add(dst64[32:64, :], t3, t4)

            # ---- Phase A + B --------------------------------------------
            with (
                tc.tile_pool(name="pa", bufs=1) as pa,
                tc.tile_pool(name="pb", bufs=1) as pb,
                tc.tile_pool(name="pbc", bufs=2) as pbc,
                tc.tile_pool(name="psA", bufs=4, space="PSUM") as psA,
                tc.tile_pool(name="psKR", bufs=1, space="PSUM") as psKR,
                tc.tile_pool(name="psB", bufs=3, space="PSUM") as psB,
            ):
                def emit_kr():
                    """k_rope for my panel; small gather (after A0, before C)"""
                    _mark(nc, "KR")
                    wkr_sb = pa.tile([P, KT_D, RD], bf16, tag="wkr")
                    xkrr = xkr.rearrange("(c k p) s -> p c k s", p=P, k=4)
                    ckr_sb = pa.tile([32, PAN], f32, tag="ckr")
                    skr_sb = pa.tile([32, PAN], f32, tag="skr")
                    xkr_ch = []
                    with tc.tile_wait_until(0.012):
                        nc.sync.dma_start(wkr_sb[:], wkr.rearrange("p (kt m) -> p kt m", kt=KT_D))
                        for c in range(4):
                            t = panels.tile([P, 4, PAN], bf16, tag="panel", name=f"xkr{c}")
                            nc.sync.dma_start(t[:], xkrr[:, c, :, :])
                            xkr_ch.append(t)
                        nc.sync.dma_start(ckr_sb[:], coskr[:])
                        nc.sync.dma_start(skr_sb[:], sinkr[:])
                    ps = psKR.tile([64, PAN], f32, tag="psKR", bufs=1, name="psKR")
                    for kt in range(KT_D):
                        nc.tensor.matmul(
                            ps, lhsT=wkr_sb[:, kt, :], rhs=xkr_ch[kt // 4][:, kt % 4, :],
                            start=(kt == 0), stop=(kt == KT_D - 1),
                        )
                    krst = work.tile([64, PAN], bf16, tag="krst", bufs=1)
                    rope_block(krst, ps, ckr_sb, skr_sb)
                    nc.sync.dma_start(kr_in[:], krst)
                    nc.gpsimd.collective_compute(
                        "AllGather", mybir.AluOpType.bypass,
                        replica_groups=GROUPS,
                        ins=[kr_in.opt()], outs=[kr_out.opt()],
                    )
                    nc.sync.dma_start(
                        k_r_sb[:], kr_out.rearrange("(g d) s -> d g s", d=64)
                    )

                # -- A weights: loaded per k-tile strip, interleaved with
                #    panel-0 x slices so the first matmul is gated on ~1us of
                #    DMA, and the serial DMA device streams at the PE's pace
                wqd_sb = pa.tile([P, KT_D, QSH], bf16, tag="wqd")
                wqdr = wqd.rearrange("p (kt m) -> p kt m", kt=KT_D)
                wkvd_sb = pa.tile([P, KT_D, KSH], bf16, tag="wkvd")
                wkvdr = wkvd.rearrange("p (kt m) -> p kt m", kt=KT_D)

                def phase_a(n):
                    """my slices of c_q / c_kv for panel n, then gather.
                    kt-major: 4 psum tiles accumulate together so PE work per
                    x-slice matches its DMA arrival."""
                    _mark(nc, f"A{n}")
                    ns = slice(n * PAN, (n + 1) * PAN)
                    xr = xT[:, ns].rearrange("(kt p) s -> p kt s", p=P)
                    x_sl = []
                    for k2 in range(KT_D // 2):
                        if n == 0 and k2 % 2 == 0:
                            kt4 = slice(2 * k2, 2 * k2 + 4)
                            nc.sync.dma_start(wqd_sb[:, kt4, :], wqdr[:, kt4, :])
                            nc.sync.dma_start(wkvd_sb[:, kt4, :], wkvdr[:, kt4, :])
                        t = panels.tile([P, 2, PAN], bf16, tag="xs", bufs=8,
                                        name=f"xs{n}_{k2}")
                        nc.sync.dma_start(t[:], xr[:, 2 * k2 : 2 * k2 + 2, :])
                        x_sl.append(t)
                    accs = [psA.tile([P, PAN], f32, tag="psA", name=f"psA{n}_{m}")
                            for m in range(4)]
                    for kt in range(KT_D):
                        xsl = x_sl[kt // 2][:, kt % 2, :]
                        for m in range(3):
                            nc.tensor.matmul(
                                accs[m],
                                lhsT=wqd_sb[:, kt, m * P : (m + 1) * P],
                                rhs=xsl,
                                start=(kt == 0), stop=(kt == KT_D - 1),
                            )
                        nc.tensor.matmul(
                            accs[3], lhsT=wkvd_sb[:, kt, :], rhs=xsl,
                            start=(kt == 0), stop=(kt == KT_D - 1),
                        )
                    for m in range(4):
                        st = work.tile([P, PAN], bf16, tag="cq_st", bufs=2)
                        nc.scalar.activation(st, accs[m], AF.Copy)
                        nc.sync.dma_start(ag_in[n][m * P : (m + 1) * P, :], st)
                    nc.gpsimd.collective_compute(
                        "AllGather", mybir.AluOpType.bypass,
                        replica_groups=GROUPS,
                        ins=[ag_in[n].opt()], outs=[ag_out[n].opt()],
                    )

                def phase_b(n):
                    """up-projections for panel n from the gathered latents"""
                    _mark(nc, f"B{n}")
                    ns = slice(n * PAN, (n + 1) * PAN)
                    # gathered latents: [(gi r p), s] with r=0..2 c_q, r=3 c_kv
                    gat = ag_out[n].rearrange("(gi r p) s -> p gi r s", p=P, r=4)
                    cq_ch = []
                    for gi in range(G):
                        t = panels.tile([P, 3, PAN], bf16, tag="panel",
                                        name=f"cq_sb{n}_{gi}")
                        nc.sync.dma_start(t[:], gat[:, gi, 0:3, :])
                        cq_ch.append(t)
                    ckv_sb = pbc.tile([P, KT_KV, PAN], bf16, tag="ckv")
                    nc.sync.dma_start(ckv_sb[:], gat[:, :, 3, :])
                    # cos replicated x4, sin x3 along partitions: the sin
                    # products are computed at shifted bases so every SB+SB
                    # combine op sees equal input base partitions (HW rule)
                    cs4 = pbc.tile([P, PAN], f32, tag="cs4", bufs=2)
                    sn3 = pbc.tile([32, PAN], f32, tag="sn3", bufs=2)
                    for r in range(4):
                        nc.sync.dma_start(cs4[32 * r : 32 * r + 32, :], cosT[:, ns])
                    nc.sync.dma_start(sn3[:], sinT[:, ns])
                    for m in range(4):  # q nope heads
                        ps = psB.tile([P, PAN], f32, tag="psB")
                        for kt in range(KT_QR):
                            nc.tensor.matmul(
                                ps,
                                lhsT=wqall_sb[:, kt, m * P : (m + 1) * P],
                                rhs=cq_ch[kt // 3][:, kt % 3, :],
                                start=(kt == 0), stop=(kt == KT_QR - 1),
                            )
                        nc.scalar.activation(qn_sb[:, m, ns], ps, AF.Copy)
                    # rope heads: two heads per M=128 matmul; bulk cos/sin
                    # products on the full psum, then 4 narrow combines
                    for hp in range(HL // 2):
                        c0 = 512 + 128 * hp
                        ps = psB.tile([P, PAN], f32, tag="psB")
                        for kt in range(KT_QR):
                            nc.tensor.matmul(
                                ps,
                                lhsT=wqall_sb[:, kt, c0 : c0 + 128],
                                rhs=cq_ch[kt // 3][:, kt % 3, :],
                                start=(kt == 0), stop=(kt == KT_QR - 1),
                            )
                        tc_ = work.tile([P, PAN], f32, tag="rtc")
                        tsu = work.tile([96, PAN], f32, tag="rtsu")
                        tsl = work.tile([P, PAN], f32, tag="rtsl")
                        nc.vector.tensor_mul(tc_, ps, cs4)
                        # t2*sin landing at base a (for out1), t1*sin at a+32;
                        # 32-partition chunks to satisfy alignment rules
                        nc.vector.tensor_mul(tsu[0:32, :], ps[32:64, :], sn3[0:32, :])
                        nc.vector.tensor_mul(tsu[64:96, :], ps[96:128, :], sn3[0:32, :])
                        nc.vector.tensor_mul(tsl[32:64, :], ps[0:32, :], sn3[0:32, :])
                        nc.vector.tensor_mul(tsl[96:128, :], ps[64:96, :], sn3[0:32, :])
                        for hh in range(2):
                            a = 64 * hh
                            h = 2 * hp + hh
                            nc.vector.tensor_sub(
                                qr_sb[0:32, h, ns], tc_[a : a + 32, :],
                                tsu[a : a + 32, :],
                            )
                            nc.vector.tensor_add(
                                qr_sb[32:64, h, ns], tc_[a + 32 : a + 64, :],
                                tsl[a + 32 : a + 64, :],
                            )
                    # k_c for this panel
                    for m in range(HL):
                        ps = psB.tile([P, PAN], f32, tag="psB")
                        for kt in range(KT_KV):
                            nc.tensor.matmul(
                                ps,
                                lhsT=wku_sb[:, kt, m * P : (m + 1) * P],
                                rhs=ckv_sb[:, kt, :],
                                start=(kt == 0), stop=(kt == KT_KV - 1),
                            )
                        nc.scalar.activation(k_c_sb[:, m, ns], ps, AF.Copy)
                    # v for this panel's S-tiles
                    for sti in range(4):
                        st = 4 * n + sti
                        ps = psB.tile([P, PAN], f32, tag="psB")
                        for kt in range(KT_KV):
                            nc.tensor.matmul(
                                ps,
                                lhsT=ckv_sb[:, kt, sti * P : (sti + 1) * P],
                                rhs=wvu_sb[:, kt, :],
                                start=(kt == 0), stop=(kt == KT_KV - 1),
                            )
                        nc.scalar.activation(v_sb[:, st, :], ps, AF.Copy)

                phase_a(0)
                # -- B weights (gpsimd DMA queue, sliced + deferred so they
                #    never block the serial DMA device during phase A)
                wqall_sb = pb.tile([P, KT_QR, 768], bf16, tag="wqall")
                wqallr = wqall.rearrange("p (kt m) -> p kt m", kt=KT_QR)
                wku_sb = pb.tile([P, KT_KV, 512], bf16, tag="wku")
                wvu_sb = pb.tile([P, KT_KV, 512], bf16, tag="wvu")
                with tc.tile_wait_until(0.02):
                    for k3 in range(KT_QR // 3):
                        kt3 = slice(3 * k3, 3 * k3 + 3)
                        nc.gpsimd.dma_start(wqall_sb[:, kt3, :], wqallr[:, kt3, :])
                    nc.gpsimd.dma_start(wku_sb[:], wku.rearrange("p (kt m) -> p kt m", kt=KT_KV))
                    nc.gpsimd.dma_start(wvu_sb[:], wvu.rearrange("p (kt m) -> p kt m", kt=KT_KV))
                phase_a(1)
                emit_kr()
                phase_a(2)
                phase_a(3)
                phase_b(0)
                phase_b(1)
                phase_b(2)
                phase_b(3)

            # ---------------- Phase C: SDPA + Phase D interleaved --------
            if "C" not in phases:
                # timing-partial build: consume B outputs so nothing is elided
                nc.gpsimd.dma_start(y[0:P, 0:PAN], qn_sb[:, 0, 0:PAN])
                nc.gpsimd.dma_start(y[P : 2 * P, 0:PAN], k_c_sb[:, 0, 0:PAN])
                nc.gpsimd.dma_start(y[2 * P : 3 * P, 0:PAN], v_sb[:, 0, 0:PAN])
                nc.gpsimd.dma_start(y[3 * P : 3 * P + 64, 0:PAN], qr_sb[:, 0, 0:PAN])
                nc.gpsimd.dma_start(y[4 * P : 4 * P + 64, 0:PAN], k_r_sb[:, 0, :])
                continue
            with (
                tc.tile_pool(name="pe", bufs=4) as pe,
                tc.tile_pool(name="pacc", bufs=2) as pacc,
                tc.tile_pool(name="pao", bufs=2) as pao,
                tc.tile_pool(name="pd", bufs=1) as pd,
                tc.tile_pool(name="pyst", bufs=3) as pyst,
                tc.tile_pool(name="psS", bufs=3, space="PSUM") as psS,
                tc.tile_pool(name="psO", bufs=2, space="PSUM") as psO,
                tc.tile_pool(name="psN", bufs=1, space="PSUM") as psN,
                tc.tile_pool(name="psD", bufs=2, space="PSUM") as psD,
            ):
                wo_sb = pd.tile([P, HL, D], bf16, tag="wo")
                wor = wo.rearrange("p (kt m) -> p kt m", kt=HL)
                with tc.tile_wait_until(0.03):
                    nc.gpsimd.dma_start(ident_sb[:], ident[:])
                    nc.gpsimd.dma_start(trib_sb[:], trib[:])
                    nc.gpsimd.dma_start(onc_sb[:], onc[:])
                    for kt in range(HL):
                        nc.gpsimd.dma_start(wo_sb[:, kt, :], wor[:, kt, :])

                run_d = "D" in phases

                # block list: (g, h, kb, j) with j the diagonal index or None
                blocks = []
                for g in range(G):
                    for h in range(HL):
                        for kb in range(4 * (g + 1)):
                            j = kb - 4 * g
                            blocks.append((g, h, kb, j if j >= 0 else None))

                st_c = {"ps_o": None, "acc": None}   # current head's psum/acc
                ao_tiles = {}                        # g -> [P, HL, PAN] tile

                def emit_scores(g, h, kb, j):
                    q0 = 0 if j is None else 128 * j
                    gs = slice(g * PAN + q0, (g + 1) * PAN)
                    ps_s = psS.tile([P, PAN], f32, tag="ps_s", name="ps_s")
                    nc.tensor.matmul(
                        ps_s[:, q0:PAN], lhsT=k_c_sb[:, h, kb * P : (kb + 1) * P],
                        rhs=qn_sb[:, h, gs], start=True, stop=False,
                    )
                    nc.tensor.matmul(
                        ps_s[:, q0:PAN],
                        lhsT=k_r_sb[:, kb // 4, (kb % 4) * P : (kb % 4 + 1) * P],
                        rhs=qr_sb[:, h, gs], start=False, stop=(j is None),
                    )
                    if j is not None:
                        # causal mask as a bias matmul on the diagonal strip
                        nc.tensor.matmul(
                            ps_s[:, q0 : q0 + P], lhsT=ident_sb, rhs=trib_sb,
                            start=False, stop=True, skip_group_check=True,
                        )
                    e = pe.tile([P, PAN], bf16, tag="e", name="e")
                    nc.scalar.activation(e[:, q0:PAN], ps_s[:, q0:PAN], AF.Exp)
                    if kb == 0:
                        st_c["acc"] = pacc.tile([P, PAN], bf16, tag="eacc", name="eacc")
                        nc.vector.tensor_copy(st_c["acc"], e)
                    else:
                        nc.vector.tensor_add(
                            st_c["acc"][:, q0:PAN], st_c["acc"][:, q0:PAN],
                            e[:, q0:PAN],
                        )
                    return (g, h, kb, 0 if j is None else q0, e, st_c["acc"])

                def emit_av(blk):
                    g, h, kb, q0, e, acc = blk
                    nk = 4 * (g + 1)
                    if kb == 0:
                        st_c["ps_o"] = psO.tile([P, PAN], f32, tag="ps_o", name="ps_o")
                    nc.tensor.matmul(
                        st_c["ps_o"][:, q0:PAN] if q0 else st_c["ps_o"],
                        lhsT=v_sb[:, kb, h * P : (h + 1) * P], rhs=e[:, q0:PAN],
                        start=(kb == 0), stop=(kb == nk - 1),
                        skip_group_check=True,
                    )
                    if kb == nk - 1:
                        # head tail: denominator + normalize into ao tile
                        if h == 0:
                            ao_tiles[g] = pao.tile([P, HL, PAN], bf16, tag="ao", name="ao")
                        ps_den = psN.tile([1, PAN], f32, tag="ps_den", name="ps_den")
                        nc.tensor.matmul(ps_den, lhsT=onc_sb, rhs=acc,
                                         start=True, stop=True)
                        rc = work.tile([1, PAN], f32, tag="rc", name="rc")
                        nc.vector.reciprocal(rc, ps_den)
                        bb = work.tile([P, PAN], f32, tag="bb", name="bb")
                        nc.gpsimd.partition_broadcast(bb, rc)
                        nc.vector.tensor_mul(
                            ao_tiles[g][:, h, :], st_c["ps_o"], bb
                        )

                def make_d_chunk(g, m, nn, pool=None):
                    def emit():
                        ao = ao_tiles[g]
                        ps = (pool or psD).tile([P, PAN], f32,
                                                tag="ps_s" if pool else "psD",
                                                name="psD")
                        for kt in range(HL):
                            nc.tensor.matmul(
                                ps,
                                lhsT=ao[:, kt, (m % 4) * P : (m % 4 + 1) * P],
                                rhs=wo_sb[:, kt, nn * PAN : (nn + 1) * PAN],
                                start=(kt == 0), stop=(kt == HL - 1),
                            )
                        yst = pyst.tile([P, PAN], f32, tag="yst", name="yst")
                        # alternate the psum drain between Act and DVE so
                        # neither queue eats the full 64-copy load
                        if (m + nn) % 2 == 0:
                            nc.scalar.activation(yst, ps, AF.Copy)
                        else:
                            nc.vector.tensor_copy(yst, ps)
                        nc.sync.dma_start(
                            y[m * P : (m + 1) * P, nn * PAN : (nn + 1) * PAN], yst
                        )
                    return emit

                fillers = []   # (ready_step, closure)
                pq = []        # blocks awaiting their attn@v (2-deep pipeline)

                def retire(blk, i):
                    emit_av(blk)
                    pg, ph, pkb = blk[0], blk[1], blk[2]
                    if ph == HL - 1 and pkb == 4 * (pg + 1) - 1 and run_d:
                        last = pg == G - 1
                        for ci, (m, nn) in enumerate(
                            (m, nn) for m in range(4 * pg, 4 * pg + 4)
                            for nn in range(D // PAN)
                        ):
                            pool = psS if (last and ci % 2) else None
                            fillers.append((i + 3, make_d_chunk(pg, m, nn, pool)))

                for i, (g, h, kb, j) in enumerate(blocks):
                    if kb == 0:
                        _mark(nc, f"C{g}h{h}")
                    pq.append(emit_scores(g, h, kb, j))
                    if fillers and fillers[0][0] <= i:
                        fillers.pop(0)[1]()
                    if len(pq) > 2:
                        retire(pq.pop(0), i)
                _mark(nc, "Cflush")
                for blk in pq:
                    retire(blk, len(blocks))
                _mark(nc, "Dtail")
                for _, f in fillers:
                    f()

    nc.compile()
    return nc


def _prep_inputs(x, positions, Wq_down, Wq_up, Wq_rope, Wkv_down, Wk_up, Wv_up,
                 Wk_rope, Wo):
    scale = np.float32(QK_D ** -0.5)
    bf = lambda a: np.ascontiguousarray(a).astype(BF16)

    def pk(a):
        # [K, M] -> [128, (K//128)*M] on-chip packed layout
        a = np.asarray(a)
        kt = a.shape[0] // P
        return bf(a.reshape(kt, P, -1).transpose(1, 0, 2).reshape(P, -1))

    shared = {
        "wkr": pk(Wk_rope.T),
        "onc": np.ones((P, 1), BF16),
        "ident": np.eye(P, dtype=np.float32).astype(BF16),
    }
    p_i = np.arange(P)[:, None]
    q_i = np.arange(P)[None, :]
    shared["trib"] = ((p_i > q_i) * np.float32(-30000.0)).astype(BF16)
    inv_freq = 1.0 / (10000.0 ** (np.arange(0, RD, 2, dtype=np.float32) / RD))
    ang = positions.astype(np.float32)[:, None] * inv_freq  # (S, 32)
    cosT = np.ascontiguousarray(np.cos(ang).T).astype(np.float32)
    sinT = np.ascontiguousarray(np.sin(ang).T).astype(np.float32)
    shared["cosT"] = cosT
    shared["sinT"] = sinT

    wqdT = Wq_down.T  # (D, QR)
    wkvdT = Wkv_down.T  # (D, KVR)
    per_g = []
    for g in range(G):
        rs, rr = slice(512 * g, 512 * (g + 1)), slice(256 * g, 256 * (g + 1))
        per_g.append({
            "wqd": pk(wqdT[:, QSH * g : QSH * (g + 1)]),
            "wkvd": pk(wkvdT[:, KSH * g : KSH * (g + 1)]),
            "wqall": pk(np.concatenate(
                [(Wq_up[rs] * scale).T, (Wq_rope[rr] * scale).T], axis=1)),
            "wku": pk(Wk_up[rs].T),
            "wvu": pk(Wv_up[rs].T),
            "wo": pk(Wo[:, rs].T),
            "coskr": np.ascontiguousarray(cosT[:, PAN * g : PAN * (g + 1)]),
            "sinkr": np.ascontiguousarray(sinT[:, PAN * g : PAN * (g + 1)]),
        })
    xT = [bf(x[b].T) for b in range(B)]

    in_maps = []
    for c in range(8):
        b, g = c // G, c % G
        m = dict(shared)
        m.update(per_g[g])
        m["xT"] = xT[b]
        m["xkr"] = np.ascontiguousarray(xT[b][:, PAN * g : PAN * (g + 1)])
        in_maps.append(m)
    return in_maps


def kernel(**inputs):
    from concourse.bass_utils import run_bass_kernel_spmd

    if "nc" not in _cache:
        _cache["nc"] = _build_module()
    nc = _cache["nc"]

    in_maps = _prep_inputs(**inputs)
    res = None
    for attempt in range(3):
        try:
            res = run_bass_kernel_spmd(nc, in_maps, core_ids=list(range(8)))
            break
        except Exception:
            if attempt == 2:
                raise
    out = np.zeros((B, S, D), np.float32)
    for c in range(8):
        out[c // G] += res.results[c]["y"]
    return out
